# revision 11
# baseline (speedup 1.0000x reference)
"""Trainium2 Bass kernel for nn_PositionEncoder (gnn_message_passing).

Strategy (8 NeuronCores, SPMD):
  - Nodes (N=2048) row-sharded 256/core. node_edge_matrix A and its
    transpose live in SBUF per-core; edge weights w computed redundantly
    per-core from a host-gathered uev matrix (pure input layout prep).
  - e = A diag(w) A^T is never materialized: per layer, z = A^T [q|sq]
    is computed as a row-sharded partial GEMM and AllReduced (1 MB);
    y = [e@q | e@sq] then comes from the w-scaled A^T shard.
  - The LSTM PQ-encoder is inherently sequential (2048 dependent steps);
    it is executed redundantly on every core (no broadcast needed),
    with gates in a [64, 4] column layout: 5 small PE matmuls and an
    ACT-only elementwise chain per step.
  - Per-molecule losses/outputs are computed as local partials and
    AllReduced once at the end.
"""

import numpy as np

import concourse.bacc as bacc
import concourse.bass as bass
import concourse.mybir as mybir
import concourse.tile as tile
from concourse.bass_utils import run_bass_kernel_spmd

F32 = mybir.dt.float32
ALU = mybir.AluOpType
ACTF = mybir.ActivationFunctionType

C = 8            # cores
N = 2048         # nodes
E = 8192         # edges
M = 128          # molecules
D = 128          # node feature dim
ED = 64          # edge feature dim
PQ = 32
H = 64           # lstm hidden (2*PQ)
TAU = 0.25
LAYERS = 4
NS = N // C      # nodes per core (256)
NT = N // 128    # node tiles total (16)
ET = E // 128    # edge tiles (64)
ZW = PQ + 1      # 33 columns in z / y

LSTM_STEPS = N   # full sequence
PHASES = 5       # build bisection knob (5 = full kernel)


def _build(nc):
    ones_np = None  # silence lint

    # ---------------- DRAM I/O ----------------
    def din(name, shape):
        return nc.dram_tensor(name, list(shape), F32, kind="ExternalInput")

    A0 = din("A0", [128, E])
    A1 = din("A1", [128, E])
    ATi = din("ATi", [E, NS])
    uevT0 = din("uevT0", [D, E])
    uevT1 = din("uevT1", [ED, E])
    uevT2 = din("uevT2", [D, E])
    vT = din("vT", [D, N])
    vTi = din("vTi", [D, NS])
    molT = din("molT", [N, M])
    molMy = din("molMy", [M, NS])
    molTi = din("molTi", [NS, M])
    selT = din("selT", [N, NS])
    colsum = din("colsum", [128, ET])
    We0 = din("We0", [D, 1]); We1 = din("We1", [ED, 1]); We2 = din("We2", [D, 1])
    be_bc = din("be_bc", [128, 1])
    Wih = [din(f"Wih{j}", [D, H]) for j in range(4)]
    Whh = [din(f"Whh{j}", [H, H]) for j in range(4)]
    bl = [din(f"bl{j}", [H, 1]) for j in range(4)]
    Wm = din("Wm", [D, 1]); bm_bc = din("bm_bc", [128, 1])
    Wd = din("Wd", [D, 1]); bd_bc = din("bd_bc", [128, 1])
    Wu1v = din("Wu1v", [D, 128])
    Wu1q = din("Wu1q", [PQ, 128])
    Wu1qT = din("Wu1qT", [128, PQ])
    Wu2 = din("Wu2", [128, 1])
    bu1 = din("bu1", [128, 1])
    bu2_bc = din("bu2_bc", [128, 1])
    I64 = din("I64", [H, H])
    ones = din("ones", [128, 1])

    p_out = nc.dram_tensor("p_out", [NS, PQ], F32, kind="ExternalOutput")
    q_out = nc.dram_tensor("q_out", [NS, PQ], F32, kind="ExternalOutput")
    sc_out = nc.dram_tensor("sc_out", [1, 2], F32, kind="ExternalOutput")
    hd_out = nc.dram_tensor("hd_out", [M, 2], F32, kind="ExternalOutput")

    RG = [list(range(C))]

    with tile.TileContext(nc) as tc:
        with (
            tc.tile_pool(name="persist", bufs=1) as pp,
            tc.tile_pool(name="dram", bufs=1, space="DRAM") as dp,
        ):
            # ---------- persistent SBUF ----------
            A0s = pp.tile([128, E], F32, tag="A0s")
            A1s = pp.tile([128, E], F32, tag="A1s")
            ATws = pp.tile([128, ET * NS], F32, tag="ATws")
            vTis = pp.tile([D, NS], F32, tag="vTis")
            molMys = pp.tile([M, NS], F32, tag="molMys")
            molTis = pp.tile([128, 2 * M], F32, tag="molTis")  # 2 chunks of [128, M]
            colsum_s = pp.tile([128, ET], F32, tag="colsum_s")
            w_col = pp.tile([128, ET], F32, tag="w_col")
            wc_col = pp.tile([128, ET], F32, tag="wc_col")
            deg_s = pp.tile([128, 2], F32, tag="deg_s")
            halfdeg_s = pp.tile([128, 2], F32, tag="halfdeg_s")
            mass_s = pp.tile([128, 2], F32, tag="mass_s")
            hmass_s = pp.tile([128, 2], F32, tag="hmass_s")
            lam_s = pp.tile([128, 2], F32, tag="lam_s")
            hl_s = pp.tile([128, 2], F32, tag="hl_s")
            m1_s = pp.tile([128, 2], F32, tag="m1_s")
            zsb = pp.tile([128, ET * ZW], F32, tag="zsb")
            stash = pp.tile([128, 4 * PQ + 6], F32, tag="stash")
            qT_s = pp.tile([PQ, NS], F32, tag="qT_s")
            eq_sb = [pp.tile([128, ZW], F32, tag=f"eq_sb{b}", name=f"eq_sb{b}") for b in range(2)]
            # state (double buffered manually)
            p_t = [[pp.tile([128, PQ], F32, tag=f"p{b}_{k}", name=f"p{b}_{k}") for b in range(2)]
                   for k in range(2)]
            q_t = [[pp.tile([128, PQ], F32, tag=f"q{b}_{k}", name=f"q{b}_{k}") for b in range(2)]
                   for k in range(2)]

            # small params
            def ptile(dr, shape, tag):
                t = pp.tile(list(shape), F32, tag=tag, name=tag)
                nc.sync.dma_start(t[:], dr[:])
                return t

            We0s = ptile(We0, [D, 1], "We0s"); We1s = ptile(We1, [ED, 1], "We1s")
            We2s = ptile(We2, [D, 1], "We2s"); bes = ptile(be_bc, [128, 1], "bes")
            Wihs = [ptile(Wih[j], [D, H], f"Wihs{j}") for j in range(4)]
            Whhs = [ptile(Whh[j], [H, H], f"Whhs{j}") for j in range(4)]
            bls = [ptile(bl[j], [H, 1], f"bls{j}") for j in range(4)]
            Wms = ptile(Wm, [D, 1], "Wms"); bms = ptile(bm_bc, [128, 1], "bms")
            Wds = ptile(Wd, [D, 1], "Wds"); bds = ptile(bd_bc, [128, 1], "bds")
            Wu1vs = ptile(Wu1v, [D, 128], "Wu1vs")
            Wu1qs = ptile(Wu1q, [PQ, 128], "Wu1qs")
            Wu1qTs = ptile(Wu1qT, [128, PQ], "Wu1qTs")
            Wu2s = ptile(Wu2, [128, 1], "Wu2s")
            bu1s = ptile(bu1, [128, 1], "bu1s")
            bu2s = ptile(bu2_bc, [128, 1], "bu2s")
            I64s = ptile(I64, [H, H], "I64s")
            ones_s = ptile(ones, [128, 1], "ones_s")

            # big persistent loads
            nc.sync.dma_start(A0s[:], A0[:])
            nc.sync.dma_start(A1s[:], A1[:])
            for t in range(ET):
                nc.sync.dma_start(ATws[:, t * NS:(t + 1) * NS],
                                  ATi[t * 128:(t + 1) * 128, :])
            nc.sync.dma_start(vTis[:], vTi[:])
            nc.sync.dma_start(molMys[:], molMy[:])
            for b in range(2):
                nc.sync.dma_start(molTis[:, b * M:(b + 1) * M],
                                  molTi[b * 128:(b + 1) * 128, :])
            nc.sync.dma_start(colsum_s[:], colsum[:])

            # ================= PHASE 1: w, ATw, deg, mass/lam =================
            with (
                tc.tile_pool(name="ph1", bufs=3) as p1,
                tc.tile_pool(name="ph1ps", bufs=1, space="PSUM") as p1ps,
            ):
                for t in range(ET):
                    u0 = p1.tile([D, 128], F32, tag="u0")
                    u1 = p1.tile([ED, 128], F32, tag="u1")
                    u2 = p1.tile([D, 128], F32, tag="u2")
                    nc.sync.dma_start(u0[:], uevT0[:, t * 128:(t + 1) * 128])
                    nc.sync.dma_start(u1[:], uevT1[:, t * 128:(t + 1) * 128])
                    nc.sync.dma_start(u2[:], uevT2[:, t * 128:(t + 1) * 128])
                    wp = p1ps.tile([128, 1], F32, tag="wp")
                    nc.tensor.matmul(wp[:], u0[:], We0s[:], start=True, stop=False)
                    nc.tensor.matmul(wp[:], u1[:], We1s[:], start=False, stop=False)
                    nc.tensor.matmul(wp[:], u2[:], We2s[:], start=False, stop=True)
                    nc.scalar.activation(w_col[:, t:t + 1], wp[:], ACTF.Sigmoid,
                                         bias=bes[:], scale=1.0)
                # wc = w * colsum
                nc.vector.tensor_tensor(wc_col[:], w_col[:], colsum_s[:], ALU.mult)
                # scale AT by w (in place)
                for t in range(ET):
                    nc.vector.tensor_scalar_mul(
                        ATws[:, t * NS:(t + 1) * NS],
                        ATws[:, t * NS:(t + 1) * NS],
                        w_col[:, t:t + 1])
                # deg
                degp = p1ps.tile([128, 2], F32, tag="degp")
                for b in range(2):
                    for t in range(ET):
                        nc.tensor.matmul(
                            degp[:, b:b + 1],
                            ATws[:, t * NS + b * 128: t * NS + b * 128 + 128],
                            colsum_s[:, t:t + 1],
                            start=(t == 0), stop=(t == ET - 1))
                nc.vector.tensor_copy(deg_s[:], degp[:])
                nc.vector.tensor_scalar_mul(halfdeg_s[:], deg_s[:], 0.5)
                # mass / lam
                mp = p1ps.tile([128, 2], F32, tag="mp")
                lp = p1ps.tile([128, 2], F32, tag="lp")
                for b in range(2):
                    nc.tensor.matmul(mp[:, b:b + 1],
                                     vTis[:, b * 128:(b + 1) * 128], Wms[:],
                                     start=True, stop=True)
                    nc.tensor.matmul(lp[:, b:b + 1],
                                     vTis[:, b * 128:(b + 1) * 128], Wds[:],
                                     start=True, stop=True)
                # softplus(x) = ln(1 + exp(x)); Softplus has no ACT table here
                expm = p1.tile([128, 2], F32, tag="expm")
                nc.scalar.activation(expm[:], mp[:], ACTF.Exp,
                                     bias=bms[:], scale=1.0)
                expm1 = p1.tile([128, 2], F32, tag="expm1")
                nc.vector.tensor_scalar_add(expm1[:], expm[:], 1.0)
                nc.scalar.activation(mass_s[:], expm1[:], ACTF.Ln)
                nc.scalar.activation(lam_s[:], lp[:], ACTF.Sigmoid,
                                     bias=bds[:], scale=1.0)
                nc.vector.tensor_scalar_add(m1_s[:], mass_s[:], -1.0)
                nc.vector.tensor_scalar_mul(hl_s[:], lam_s[:], 0.5)
                nc.vector.tensor_scalar_mul(hmass_s[:], mass_s[:], 0.5)

            # ================= PHASE 2: LSTM =================
            with (
                tc.tile_pool(name="lstm", bufs=1) as lp_,
                tc.tile_pool(name="lstmw", bufs=2) as lw,
                tc.tile_pool(name="lstm_state", bufs=2) as ls,
            ):
                xwb4 = lp_.tile([H, 4 * N], F32, tag="xwb4")
                HS = lp_.tile([H, N], F32, tag="HS")
                nc.vector.memset(HS[:], 0.0)
                xwb4_v = xwb4.rearrange("p (t j) -> p t j", j=4)
                with (
                    tc.tile_pool(name="lstm_ps", bufs=2, space="PSUM") as lps,
                    tc.tile_pool(name="xwb_ps", bufs=2, space="PSUM") as xps,
                ):
                    # xwb: per gate j over 4 chunks of 512 steps
                    for ch in range(4):
                        vt = lw.tile([D, 512], F32, tag="vt")
                        nc.sync.dma_start(vt[:], vT[:, ch * 512:(ch + 1) * 512])
                        for j in range(4):
                            xp = xps.tile([H, 512], F32, tag="xp")
                            nc.tensor.matmul(xp[:], Wihs[j][:], vt[:],
                                             start=True, stop=True)
                            nc.scalar.activation(
                                xwb4_v[:, ch * 512:(ch + 1) * 512, j],
                                xp[:], ACTF.Identity, bias=bls[j][:], scale=1.0)

                    cz = ls.tile([H, 1], F32, tag="c0init")
                    hz = ls.tile([H, 1], F32, tag="h0init")
                    nc.vector.memset(cz[:], 0.0)
                    nc.vector.memset(hz[:], 0.0)
                    c_prev, h_prev = cz, hz
                    for t in range(LSTM_STEPS):
                        g = lps.tile([H, 4], F32, tag="g")
                        nc.tensor.matmul(g[:], I64s[:], xwb4[:, 4 * t:4 * t + 4],
                                         start=True, stop=False)
                        for j in range(4):
                            nc.tensor.matmul(g[:, j:j + 1], Whhs[j][:], h_prev[:],
                                             start=False, stop=(j == 3))
                        s3 = ls.tile([H, 3], F32, tag="s3")
                        tg = ls.tile([H, 1], F32, tag="tg")
                        nc.scalar.activation(s3[:], g[:, 0:3], ACTF.Sigmoid)
                        nc.scalar.activation(tg[:], g[:, 3:4], ACTF.Tanh)
                        t1 = ls.tile([H, 1], F32, tag="t1")
                        nc.scalar.activation(t1[:], tg[:], ACTF.Copy,
                                             scale=s3[:, 0:1])
                        c_new = ls.tile([H, 1], F32, tag="c")
                        nc.scalar.activation(c_new[:], c_prev[:], ACTF.Identity,
                                             bias=t1[:], scale=s3[:, 1:2])
                        tc_ = ls.tile([H, 1], F32, tag="tc")
                        nc.scalar.activation(tc_[:], c_new[:], ACTF.Tanh)
                        h_new = ls.tile([H, 1], F32, tag="h")
                        nc.scalar.activation(h_new[:], tc_[:], ACTF.Copy,
                                             scale=s3[:, 2:3])
                        nc.vector.tensor_copy(HS[:, t:t + 1], h_new[:])
                        c_prev, h_prev = c_new, h_new

                # ============= PHASE 3: centering + state extraction =============
                with (
                    tc.tile_pool(name="cent", bufs=1) as cp,
                    tc.tile_pool(name="cent2", bufs=2) as cp2,
                    tc.tile_pool(name="cent_ps", bufs=2, space="PSUM") as cps,
                    tc.tile_pool(name="cent_ps1", bufs=1, space="PSUM") as cps1,
                ):
                    molT_sb = cp.tile([128, NT * M], F32, tag="molT_sb")
                    for ch in range(NT):
                        nc.sync.dma_start(molT_sb[:, ch * M:(ch + 1) * M],
                                          molT[ch * 128:(ch + 1) * 128, :])
                    hsnm = cp.tile([128, NT * H], F32, tag="hsnm")
                    for ch in range(NT):
                        tp = cps.tile([128, H], F32, tag="tr")
                        nc.tensor.transpose(tp[:], HS[:, ch * 128:(ch + 1) * 128],
                                            I64s[:])
                        nc.vector.tensor_copy(hsnm[:, ch * H:(ch + 1) * H], tp[:])
                    # counts
                    cnt_ps = cps1.tile([128, 1], F32, tag="cnt")
                    for ch in range(NT):
                        nc.tensor.matmul(cnt_ps[:],
                                         molT_sb[:, ch * M:(ch + 1) * M],
                                         ones_s[:],
                                         start=(ch == 0), stop=(ch == NT - 1))
                    cnt = cp2.tile([128, 1], F32, tag="cnts")
                    nc.vector.tensor_scalar_max(cnt[:], cnt_ps[:], 1.0)
                    invc = cp2.tile([128, 1], F32, tag="invc")
                    nc.vector.reciprocal(invc[:], cnt[:])
                    # mqT = (mol @ q0)^T  [PQ, M]
                    mqT_ps = cps1.tile([PQ, M], F32, tag="mqT")
                    for ch in range(NT):
                        nc.tensor.matmul(mqT_ps[:],
                                         hsnm[:, ch * H + PQ: ch * H + H],
                                         molT_sb[:, ch * M:(ch + 1) * M],
                                         start=(ch == 0), stop=(ch == NT - 1))
                    mqT_sb = cp2.tile([PQ, M], F32, tag="mqTs")
                    nc.vector.tensor_copy(mqT_sb[:], mqT_ps[:])
                    mq_ps = cps.tile([M, PQ], F32, tag="tr")
                    nc.tensor.transpose(mq_ps[:], mqT_sb[:], I64s[0:PQ, 0:PQ])
                    mq_sb = cp2.tile([M, PQ], F32, tag="mqs")
                    nc.vector.tensor_scalar_mul(mq_sb[:], mq_ps[:], invc[:])
                    # extract my p/q via selection matmul
                    pq_ps = [cps.tile([128, H], F32, tag="pq", name=f"pq_ps{_b}") for _b in range(2)]
                    for ch in range(NT):
                        st = cp2.tile([128, NS], F32, tag="selch")
                        nc.sync.dma_start(
                            st[:], selT[ch * 128:(ch + 1) * 128, :])
                        for b in range(2):
                            nc.tensor.matmul(
                                pq_ps[b][:],
                                st[:, b * 128:(b + 1) * 128],
                                hsnm[:, ch * H:(ch + 1) * H],
                                start=(ch == 0), stop=(ch == NT - 1))
                    for b in range(2):
                        nc.vector.tensor_copy(p_t[0][b][:], pq_ps[b][:, 0:PQ])
                        qu = cp2.tile([128, PQ], F32, tag="qu")
                        nc.vector.tensor_copy(qu[:], pq_ps[b][:, PQ:H])
                        csub = cps.tile([128, PQ], F32, tag="tr")
                        nc.tensor.matmul(csub[:],
                                         molMys[:, b * 128:(b + 1) * 128],
                                         mq_sb[:], start=True, stop=True)
                        nc.vector.tensor_tensor(q_t[0][b][:], qu[:], csub[:],
                                                ALU.subtract)

            # qT for layer 0
            for b in range(2):
                for r in range(4):
                    nc.vector.transpose(
                        qT_s[0:PQ, b * 128 + r * PQ: b * 128 + (r + 1) * PQ],
                        q_t[0][b][r * PQ:(r + 1) * PQ, 0:PQ])

            # ================= PHASE 4: layers =================
            z_in = dp.tile([128, ET * ZW], F32, tag="z_in")
            z_outd = dp.tile([128, ET * ZW], F32, tag="z_out")
            st_in = dp.tile([128, 4 * PQ + 6], F32, tag="st_in")
            st_outd = dp.tile([128, 4 * PQ + 6], F32, tag="st_out")

            with (
                tc.tile_pool(name="lay", bufs=2) as lyp,
                tc.tile_pool(name="lay_zps", bufs=3, space="PSUM") as zps,
                tc.tile_pool(name="lay_yps", bufs=2, space="PSUM") as yps,
                tc.tile_pool(name="lay_mps", bufs=1, space="PSUM") as mps,
            ):
                for l in range(LAYERS):
                    cur, nxt = l % 2, (l + 1) % 2
                    p_c, q_c = p_t[cur], q_t[cur]
                    p_n, q_n = p_t[nxt], q_t[nxt]
                    last = (l == LAYERS - 1)

                    # qsq = [q | sq]
                    qsq = [lyp.tile([128, ZW], F32, tag=f"qsq{b}", name=f"qsq{b}") for b in range(2)]
                    for b in range(2):
                        scr = lyp.tile([128, PQ], F32, tag="scr")
                        nc.vector.tensor_tensor(scr[:], q_c[b][:], q_c[b][:],
                                                ALU.mult)
                        nc.vector.tensor_reduce(qsq[b][:, PQ:PQ + 1], scr[:],
                                                mybir.AxisListType.X, ALU.add)
                        nc.vector.tensor_copy(qsq[b][:, 0:PQ], q_c[b][:])
                    # GEMM1: z partial
                    for t in range(ET):
                        zp = zps.tile([128, ZW], F32, tag="z")
                        nc.tensor.matmul(zp[:], A0s[:, t * 128:(t + 1) * 128],
                                         qsq[0][:], start=True, stop=False)
                        nc.tensor.matmul(zp[:], A1s[:, t * 128:(t + 1) * 128],
                                         qsq[1][:], start=False, stop=True)
                        if t % 2 == 0:
                            nc.scalar.copy(zsb[:, t * ZW:(t + 1) * ZW], zp[:])
                        else:
                            nc.vector.tensor_copy(zsb[:, t * ZW:(t + 1) * ZW], zp[:])
                    # AllReduce z
                    nc.sync.dma_start(z_in[:], zsb[:])
                    nc.gpsimd.collective_compute(
                        "AllReduce", ALU.add, replica_groups=RG,
                        ins=[z_in.opt()], outs=[z_outd.opt()])
                    nc.sync.dma_start(zsb[:], z_outd[:])
                    # GEMM2: y = [e@q | e@sq] for my nodes
                    for b in range(2):
                        yp = yps.tile([128, ZW], F32, tag="y")
                        for t in range(ET):
                            nc.tensor.matmul(
                                yp[:],
                                ATws[:, t * NS + b * 128: t * NS + b * 128 + 128],
                                zsb[:, t * ZW:(t + 1) * ZW],
                                start=(t == 0), stop=(t == ET - 1))
                        nc.vector.tensor_copy(eq_sb[b][:], yp[:])
                    # MLP
                    hp = mps.tile([128, NS], F32, tag="hp")
                    nc.tensor.matmul(hp[:], Wu1vs[:], vTis[:], start=True, stop=False)
                    nc.tensor.matmul(hp[:], Wu1qs[:], qT_s[:], start=False, stop=True)
                    ua = lyp.tile([128, NS], F32, tag="ua")
                    nc.scalar.activation(ua[:], hp[:], ACTF.Relu,
                                         bias=bu1s[:], scale=1.0)
                    mw = lyp.tile([128, NS], F32, tag="mw")
                    nc.vector.tensor_scalar(mw[:], ua[:], 0.0, Wu2s[:],
                                            ALU.is_gt, ALU.mult)
                    gqu = [yps.tile([128, PQ], F32, tag="y", name=f"gqu{_b}") for _b in range(2)]
                    for b in range(2):
                        nc.tensor.matmul(gqu[b][:],
                                         mw[:, b * 128:(b + 1) * 128],
                                         Wu1qTs[:], start=True, stop=True)
                    # assembly
                    dvals = []
                    for b in range(2):
                        dq = lyp.tile([128, PQ], F32, tag=f"dq{b}")
                        nc.vector.tensor_scalar_mul(dq[:], p_c[b][:],
                                                    mass_s[:, b:b + 1])
                        t1_ = lyp.tile([128, PQ], F32, tag=f"t1_{b}")
                        nc.vector.scalar_tensor_tensor(
                            t1_[:], q_c[b][:], deg_s[:, b:b + 1],
                            eq_sb[b][:, 0:PQ], ALU.mult, ALU.subtract)
                        t2_ = lyp.tile([128, PQ], F32, tag=f"t2_{b}")
                        nc.vector.scalar_tensor_tensor(
                            t2_[:], t1_[:], -2.0, gqu[b][:],
                            ALU.mult, ALU.subtract)
                        pl = lyp.tile([128, PQ], F32, tag=f"pl{b}")
                        nc.vector.scalar_tensor_tensor(
                            pl[:], p_c[b][:], lam_s[:, b:b + 1], t2_[:],
                            ALU.mult, ALU.subtract)
                        # p_new = p - TAU*pl ; q_new = q + TAU*dq
                        nc.vector.scalar_tensor_tensor(
                            p_n[b][:], pl[:], -TAU, p_c[b][:], ALU.mult, ALU.add)
                        nc.vector.scalar_tensor_tensor(
                            q_n[b][:], dq[:], TAU, q_c[b][:], ALU.mult, ALU.add)
                        d_ = lyp.tile([128, PQ], F32, tag=f"d{b}")
                        nc.vector.tensor_tensor(d_[:], p_n[b][:], p_c[b][:],
                                                ALU.subtract)
                        dvals.append(d_)
                    # s_loss partial
                    sscr = lyp.tile([128, PQ], F32, tag="sscr")
                    ss0 = lyp.tile([128, 1], F32, tag="ss0")
                    t3 = lyp.tile([128, PQ], F32, tag="t3")
                    nc.vector.tensor_scalar_mul(t3[:], p_c[0][:], m1_s[:, 0:1])
                    nc.vector.tensor_tensor(sscr[:], t3[:], t3[:], ALU.mult)
                    nc.vector.tensor_reduce(ss0[:], sscr[:],
                                            mybir.AxisListType.X, ALU.add)
                    t4 = lyp.tile([128, PQ], F32, tag="t4")
                    ss1 = lyp.tile([128, 1], F32, tag="ss1")
                    sscr2 = lyp.tile([128, PQ], F32, tag="sscr2")
                    nc.vector.tensor_scalar_mul(t4[:], p_c[1][:], m1_s[:, 1:2])
                    nc.vector.tensor_tensor(sscr2[:], t4[:], t4[:], ALU.mult)
                    nc.vector.tensor_reduce(ss1[:], sscr2[:],
                                            mybir.AxisListType.X, ALU.add)
                    nc.vector.tensor_tensor(
                        stash[:, 4 * PQ + l:4 * PQ + l + 1], ss0[:], ss1[:],
                        ALU.add)
                    # molp partial -> stash
                    mpp = yps.tile([128, PQ], F32, tag="y")
                    for b in range(2):
                        nc.tensor.matmul(mpp[:],
                                         molTis[:, b * M:(b + 1) * M],
                                         dvals[b][:],
                                         start=(b == 0), stop=(b == 1))
                    nc.vector.tensor_copy(stash[:, l * PQ:(l + 1) * PQ], mpp[:])

                    if last:
                        # energy + h_mol/d_mol partials (uses p_c, q_c pre-update)
                        en = []
                        dmv = []
                        for b in range(2):
                            ps2 = lyp.tile([128, 1], F32, tag=f"ps2{b}")
                            escr = lyp.tile([128, PQ], F32, tag="escr")
                            nc.vector.tensor_tensor(escr[:], p_c[b][:],
                                                    p_c[b][:], ALU.mult)
                            nc.vector.tensor_reduce(ps2[:], escr[:],
                                                    mybir.AxisListType.X,
                                                    ALU.add)
                            qd = lyp.tile([128, 1], F32, tag=f"qd{b}")
                            escr2 = lyp.tile([128, PQ], F32, tag="escr2")
                            nc.vector.tensor_tensor(escr2[:], q_c[b][:],
                                                    eq_sb[b][:, 0:PQ], ALU.mult)
                            nc.vector.tensor_reduce(qd[:], escr2[:],
                                                    mybir.AxisListType.X,
                                                    ALU.add)
                            ucol = yps.tile([128, 1], F32, tag="y")
                            nc.tensor.matmul(ucol[:],
                                             ua[:, b * 128:(b + 1) * 128],
                                             Wu2s[:], start=True, stop=True)
                            e1 = lyp.tile([128, 1], F32, tag=f"e1{b}")
                            nc.vector.scalar_tensor_tensor(
                                e1[:], ps2[:], hmass_s[:, b:b + 1], qd[:],
                                ALU.mult, ALU.subtract)
                            e2 = lyp.tile([128, 1], F32, tag=f"e2{b}")
                            nc.vector.scalar_tensor_tensor(
                                e2[:], eq_sb[b][:, PQ:ZW], 0.5, e1[:],
                                ALU.mult, ALU.add)
                            e3 = lyp.tile([128, 1], F32, tag=f"e3{b}")
                            nc.vector.scalar_tensor_tensor(
                                e3[:], qsq[b][:, PQ:ZW], halfdeg_s[:, b:b + 1],
                                e2[:], ALU.mult, ALU.add)
                            e4 = lyp.tile([128, 1], F32, tag=f"e4{b}")
                            nc.vector.tensor_tensor(e4[:], e3[:], ucol[:], ALU.add)
                            e5 = lyp.tile([128, 1], F32, tag=f"e5{b}")
                            nc.vector.tensor_scalar(e5[:], e4[:], bu2s[:], None,
                                                    ALU.add)
                            en.append(e5)
                            dv = lyp.tile([128, 1], F32, tag=f"dv{b}")
                            nc.vector.tensor_scalar_mul(dv[:], ps2[:],
                                                        hl_s[:, b:b + 1])
                            dmv.append(dv)
                        hm_ps = yps.tile([128, 1], F32, tag="y")
                        for b in range(2):
                            nc.tensor.matmul(hm_ps[:],
                                             molTis[:, b * M:(b + 1) * M],
                                             en[b][:], start=(b == 0),
                                             stop=(b == 1))
                        nc.vector.tensor_copy(stash[:, 4 * PQ + 4:4 * PQ + 5],
                                              hm_ps[:])
                        dm_ps = yps.tile([128, 1], F32, tag="y")
                        for b in range(2):
                            nc.tensor.matmul(dm_ps[:],
                                             molTis[:, b * M:(b + 1) * M],
                                             dmv[b][:], start=(b == 0),
                                             stop=(b == 1))
                        nc.vector.tensor_copy(stash[:, 4 * PQ + 5:4 * PQ + 6],
                                              dm_ps[:])
                    else:
                        # qT for next layer
                        for b in range(2):
                            for r in range(4):
                                nc.vector.transpose(
                                    qT_s[0:PQ,
                                         b * 128 + r * PQ: b * 128 + (r + 1) * PQ],
                                    q_n[b][r * PQ:(r + 1) * PQ, 0:PQ])

                # ================= PHASE 5: final AR + losses =================
                nc.sync.dma_start(st_in[:], stash[:])
                nc.gpsimd.collective_compute(
                    "AllReduce", ALU.add, replica_groups=RG,
                    ins=[st_in.opt()], outs=[st_outd.opt()])
                sts = lyp.tile([128, 4 * PQ + 6], F32, tag="sts")
                nc.sync.dma_start(sts[:], st_outd[:])

                scl = lyp.tile([1, 8], F32, tag="scl")
                for l in range(LAYERS):
                    cscr = lyp.tile([128, PQ], F32, tag="cscr")
                    cs = lyp.tile([128, 1], F32, tag="cs")
                    nc.vector.tensor_tensor(cscr[:], sts[:, l * PQ:(l + 1) * PQ],
                                            sts[:, l * PQ:(l + 1) * PQ],
                                            ALU.mult)
                    nc.vector.tensor_reduce(cs[:], cscr[:],
                                            mybir.AxisListType.X, ALU.add)
                    csp = yps.tile([1, 1], F32, tag="y")
                    nc.tensor.matmul(csp[:], cs[:], ones_s[:], start=True,
                                     stop=True)
                    nc.scalar.activation(scl[0:1, 4 + l:5 + l], csp[:], ACTF.Sqrt)
                    ssp = yps.tile([1, 1], F32, tag="y")
                    nc.tensor.matmul(ssp[:],
                                     sts[:, 4 * PQ + l:4 * PQ + l + 1],
                                     ones_s[:], start=True, stop=True)
                    nc.scalar.activation(scl[0:1, l:l + 1], ssp[:], ACTF.Sqrt)
                scf = lyp.tile([1, 2], F32, tag="scf")
                ta = lyp.tile([1, 2], F32, tag="ta")
                nc.vector.tensor_tensor(ta[:, 0:1], scl[0:1, 0:1],
                                        scl[0:1, 1:2], ALU.add)
                nc.vector.tensor_tensor(ta[:, 1:2], scl[0:1, 2:3],
                                        scl[0:1, 3:4], ALU.add)
                nc.vector.tensor_tensor(scf[:, 0:1], ta[:, 0:1], ta[:, 1:2],
                                        ALU.add)
                tb = lyp.tile([1, 2], F32, tag="tb")
                nc.vector.tensor_tensor(tb[:, 0:1], scl[0:1, 4:5],
                                        scl[0:1, 5:6], ALU.add)
                nc.vector.tensor_tensor(tb[:, 1:2], scl[0:1, 6:7],
                                        scl[0:1, 7:8], ALU.add)
                nc.vector.tensor_tensor(scf[:, 1:2], tb[:, 0:1], tb[:, 1:2],
                                        ALU.add)
                nc.sync.dma_start(sc_out[:], scf[:])

                hd = lyp.tile([128, 2], F32, tag="hd")
                nc.vector.tensor_copy(hd[:], sts[:, 4 * PQ + 4:4 * PQ + 6])
                nc.sync.dma_start(hd_out[:], hd[:])

                fin = (LAYERS - 1 + 1) % 2
                for b in range(2):
                    nc.sync.dma_start(p_out[b * 128:(b + 1) * 128, :],
                                      p_t[fin][b][:])
                    nc.sync.dma_start(q_out[b * 128:(b + 1) * 128, :],
                                      q_t[fin][b][:])
    return nc


_CACHE = {}


def _get_nc():
    if "nc" not in _CACHE:
        nc = bacc.Bacc("TRN2", target_bir_lowering=False, debug=False,
                       num_devices=C)
        _build(nc)
        nc.compile()
        _CACHE["nc"] = nc
    return _CACHE["nc"]


def _prep_inputs(inputs):
    f = np.float32
    v = np.asarray(inputs["v_features"], f)
    ef = np.asarray(inputs["e_features"], f)
    us = np.asarray(inputs["us"]).astype(np.int64)
    vs = np.asarray(inputs["vs"]).astype(np.int64)
    A = np.asarray(inputs["node_edge_matrix"], f)
    molm = np.asarray(inputs["mol_node_matrix"], f)
    W_e = np.asarray(inputs["W_e"], f); b_e = np.asarray(inputs["b_e"], f)
    W_ih = np.asarray(inputs["W_ih"], f); W_hh = np.asarray(inputs["W_hh"], f)
    b_lstm = np.asarray(inputs["b_lstm"], f)
    W_m = np.asarray(inputs["W_m"], f); b_m = np.asarray(inputs["b_m"], f)
    W_d = np.asarray(inputs["W_d"], f); b_d = np.asarray(inputs["b_d"], f)
    W_u1 = np.asarray(inputs["W_u1"], f); b_u1 = np.asarray(inputs["b_u1"], f)
    W_u2 = np.asarray(inputs["W_u2"], f); b_u2 = np.asarray(inputs["b_u2"], f)

    # gate permutation: device order (i, f, o, gg)
    perm = [np.arange(0, 64), np.arange(64, 128),
            np.arange(192, 256), np.arange(128, 192)]

    uevT0 = np.ascontiguousarray(v[us].T)
    uevT1 = np.ascontiguousarray(ef.T)
    uevT2 = np.ascontiguousarray(v[vs].T)
    colsum = A.sum(0)  # [E]
    colsum_col = np.ascontiguousarray(colsum.reshape(ET, 128).T)
    vT_full = np.ascontiguousarray(v.T)
    molT = np.ascontiguousarray(molm.T)

    common = dict(
        uevT0=uevT0, uevT1=uevT1, uevT2=uevT2,
        vT=vT_full, molT=molT, colsum=colsum_col,
        We0=W_e[:D], We1=W_e[D:D + ED], We2=W_e[D + ED:],
        be_bc=np.full((128, 1), b_e[0], f),
        Wm=W_m, bm_bc=np.full((128, 1), b_m[0], f),
        Wd=W_d, bd_bc=np.full((128, 1), b_d[0], f),
        Wu1v=np.ascontiguousarray(W_u1[:D]),
        Wu1q=np.ascontiguousarray(W_u1[D:]),
        Wu1qT=np.ascontiguousarray(W_u1[D:].T),
        Wu2=W_u2, bu1=b_u1.reshape(128, 1),
        bu2_bc=np.full((128, 1), b_u2[0], f),
        I64=np.eye(H, dtype=f),
        ones=np.ones((128, 1), f),
    )
    for j in range(4):
        common[f"Wih{j}"] = np.ascontiguousarray(W_ih[:, perm[j]])
        common[f"Whh{j}"] = np.ascontiguousarray(W_hh[:, perm[j]])
        common[f"bl{j}"] = np.ascontiguousarray(b_lstm[perm[j]].reshape(H, 1))

    in_maps = []
    for i in range(C):
        rows = slice(i * NS, (i + 1) * NS)
        Ai = A[rows]
        selT = np.zeros((N, NS), f)
        selT[rows, :] = np.eye(NS, dtype=f)
        m = dict(common)
        m.update(
            A0=np.ascontiguousarray(Ai[:128]),
            A1=np.ascontiguousarray(Ai[128:]),
            ATi=np.ascontiguousarray(Ai.T),
            vTi=np.ascontiguousarray(v[rows].T),
            molMy=np.ascontiguousarray(molm[:, rows]),
            molTi=np.ascontiguousarray(molm[:, rows].T),
            selT=selT,
        )
        in_maps.append(m)
    return in_maps


def kernel(**inputs):
    nc = _get_nc()
    in_maps = _prep_inputs(inputs)
    res = run_bass_kernel_spmd(nc, in_maps, list(range(C)))
    r = res.results
    p = np.concatenate([r[i]["p_out"] for i in range(C)], 0)
    q = np.concatenate([r[i]["q_out"] for i in range(C)], 0)
    sc = r[0]["sc_out"]
    hd = r[0]["hd_out"]
    s_loss = np.float32(sc[0, 0])
    c_loss = np.float32(sc[0, 1])
    h_mol = np.ascontiguousarray(hd[:, 0])
    d_mol = np.ascontiguousarray(hd[:, 1])
    return p, q, s_loss, c_loss, h_mol, d_mol
